# revision 15
# baseline (speedup 1.0000x reference)
"""Trainium2 Bass kernel for AttnDecoderBlock (window attention + MLP + bilinear upsample).

Strategy: pure data-parallel over batch B=128 -> 8 cores x 16 samples.
On-chip layout is feature-major [C_partition, token_free].  LN scale/bias are
folded into the following GEMM weights on the host; attention uses the
S^T = k^T q orientation with exp(S)*exp(bias) and a host-precomputed
padded-key denominator correction PB[h,i] = sum_{j>=300} exp(bias[h,i,j]).
All GEMMs bf16 with fp32 PSUM accumulation; residual trunk bf16.

v6: three-stage software pipeline across groups -- front(g) = load+LN1+qk+v,
mid(g-1) = attention+proj, tail(g-2) = LN2+MLP+out+upsample -- so the PE has
ready GEMM work from adjacent groups during each group's serial LN/softmax
chains.  LN rstd via all-DVE Newton rsqrt (keeps Sqrt off the ACT engine --
table-set switches between Exp/Gelu/Sqrt cost 2.7us each).
"""

import numpy as np
import ml_dtypes
from contextlib import ExitStack

from concourse import bacc, mybir
from concourse.bass import _add_dep_helper
import concourse.bass as bass
import concourse.tile as tile

dt = mybir.dt
BF = dt.bfloat16
F32 = dt.float32
AF = mybir.ActivationFunctionType
OP = mybir.AluOpType

# problem constants (hardcoded per spec)
B, C, NH, WS, H, W = 128, 384, 6, 20, 15, 20
HD = C // NH            # 64
L = H * W               # 300 real tokens
N = WS * WS             # 400 padded tokens
OUT_DIM, OUT_H, OUT_W = 192, 30, 40
N_CORES = 8
S = B // N_CORES        # 16 samples per core
G = 2                   # samples per group (token-batch for GEMM stages)
GT = G * L              # 600
NBLK = 65               # v^T block width per head: 64 dims + 1 ones col
JCH = [(0, 128), (128, 128), (256, 44)]   # attention key/token chunks
bf16 = ml_dtypes.bfloat16


def _rel_pos_index(ws):
    coords = np.stack(np.meshgrid(np.arange(ws), np.arange(ws), indexing='ij')).reshape(2, -1)
    rel = (coords[:, :, None] - coords[:, None, :]).transpose(1, 2, 0)
    rel[:, :, 0] += ws - 1
    rel[:, :, 1] += ws - 1
    rel[:, :, 0] *= 2 * ws - 1
    return rel.sum(-1)


def build_consts(ln1_s, ln1_b, qkv_w, qkv_b, bias_table, proj_w, proj_b,
                 ln2_s, ln2_b, fc1_w, fc1_b, fc2_w, fc2_b, out_w, out_b):
    """Host-side weight folding and layout. Returns dict name -> np array."""
    f32 = np.float32
    qkv_w = np.asarray(qkv_w, f32); qkv_b = np.asarray(qkv_b, f32)
    ln1_s = np.asarray(ln1_s, f32); ln1_b = np.asarray(ln1_b, f32)
    # fold LN1 affine into qkv weights; fold attention scale into q
    Wq = ln1_s[:, None] * qkv_w[:, 0:C] * (HD ** -0.5)
    Wk = ln1_s[:, None] * qkv_w[:, C:2 * C]
    Wv = ln1_s[:, None] * qkv_w[:, 2 * C:3 * C]
    bq = (ln1_b @ qkv_w[:, 0:C] + qkv_b[0:C]) * (HD ** -0.5)
    bk = ln1_b @ qkv_w[:, C:2 * C] + qkv_b[C:2 * C]
    bv = ln1_b @ qkv_w[:, 2 * C:] + qkv_b[2 * C:]
    assert not np.any(bq) and not np.any(bk) and not np.any(bv), \
        "nonzero qkv/ln1 bias path not implemented"
    Wqk = np.concatenate([Wq, Wk], axis=1)            # [C, 768]
    W1 = np.asarray(ln2_s, f32)[:, None] * np.asarray(fc1_w, f32)
    b1 = np.asarray(ln2_b, f32) @ np.asarray(fc1_w, f32) + np.asarray(fc1_b, f32)
    assert not np.any(b1) and not np.any(proj_b) and not np.any(fc2_b) and not np.any(out_b), \
        "nonzero bias path not implemented"

    REL = _rel_pos_index(WS)
    bias = np.asarray(bias_table, f32)[REL].transpose(2, 0, 1)   # [NH, 400, 400]
    EB_T = np.exp(bias[:, :L, :L].transpose(0, 2, 1))            # [NH, j, i]
    PB = np.exp(bias[:, :L, L:]).sum(-1)                         # [NH, 300]

    bsel = np.zeros((NH, 3 * 128), f32)
    for h in range(NH):
        bsel[h, (h // 2) * 128 + (h % 2) * 64: (h // 2) * 128 + (h % 2) * 64 + 64] = 1.0
    e4 = np.zeros((G, G * 128), f32)
    for s in range(G):
        e4[s, s * 128:(s + 1) * 128] = 1.0

    c = {}
    c['wqk'] = np.ascontiguousarray(Wqk.reshape(3, 128, 2 * C)).astype(bf16)
    c['wv'] = np.ascontiguousarray(Wv.reshape(3, 128, C)).astype(bf16)
    c['wp'] = np.ascontiguousarray(np.asarray(proj_w, f32).reshape(3, 128, C)).astype(bf16)
    c['w1'] = np.ascontiguousarray(W1.reshape(3, 128, 4 * C)).astype(bf16)
    c['w2'] = np.ascontiguousarray(np.asarray(fc2_w, f32).reshape(12, 128, C)).astype(bf16)
    c['wo'] = np.ascontiguousarray(np.asarray(out_w, f32).reshape(3, 128, OUT_DIM)).astype(bf16)
    # EB^T in j-chunk layout: [jc, NH, jw<=128 partitions, 300]
    ebt = np.zeros((3, NH, 128, L), f32)
    for jc, (j0, jw) in enumerate(JCH):
        ebt[jc, :, 0:jw, :] = EB_T[:, j0:j0 + jw, :]
    c['eb'] = ebt.astype(bf16)                                   # [3, NH, 128, 300]
    c['pb'] = PB.astype(f32)                                     # [6, 300]
    c['bsel'] = bsel.astype(bf16)                                # [6, 384]
    c['e4'] = e4.astype(bf16)                                    # [G, G*128]
    c['ones_b'] = np.ones((128, 1), bf16)
    return c


CONST_SPECS = [
    ('wqk', (3, 128, 2 * C), BF), ('wv', (3, 128, C), BF), ('wp', (3, 128, C), BF),
    ('w1', (3, 128, 4 * C), BF), ('w2', (12, 128, C), BF), ('wo', (3, 128, OUT_DIM), BF),
    ('eb', (3, NH, 128, L), BF), ('pb', (NH, L), F32),
    ('bsel', (NH, 3 * 128), BF), ('e4', (G, G * 128), BF),
    ('ones_b', (128, 1), BF),
]


def build_program(n_samples, debug=False):
    """Build the Bass program for one core processing n_samples samples."""
    nc = bacc.Bacc(None, target_bir_lowering=False, debug=debug)
    xin = nc.dram_tensor("xin", [n_samples, 3, 128, L], BF, kind="ExternalInput")
    outd = nc.dram_tensor("out", [n_samples, OUT_DIM, OUT_H, OUT_W], BF,
                          kind="ExternalOutput")
    cdram = {name: nc.dram_tensor(name, list(shape), d, kind="ExternalInput")
             for name, shape, d in CONST_SPECS}

    with tile.TileContext(nc) as tc, ExitStack() as ctx:
        cpool = ctx.enter_context(tc.tile_pool(name="consts", bufs=1))
        pool = ctx.enter_context(tc.tile_pool(name="main", bufs=1))
        ps = ctx.enter_context(tc.tile_pool(name="psum", bufs=1, space="PSUM"))

        # ---- resident constants -> SBUF
        cs = {}
        for name, shape, d in CONST_SPECS:
            if len(shape) == 2:
                t = cpool.tile([shape[0] if shape[0] > 1 else 1, shape[1]], d, tag=name, name=name)
                nc.sync.dma_start(t[:], cdram[name][:])
            elif name == 'eb':
                t = cpool.tile([128, 3 * NH * L], d, tag=name, name=name)
                nc.sync.dma_start(t.rearrange("p (j h i) -> p j h i", j=3, h=NH),
                                  cdram[name].rearrange("j h p i -> p j h i"))
            else:  # [k, 128, F] weight stacks
                k, p, f = shape
                t = cpool.tile([128, k * f], d, tag=name, name=name)
                nc.sync.dma_start(t.rearrange("p (k f) -> p k f", k=k),
                                  cdram[name].rearrange("k p f -> p k f"))
            cs[name] = t

        def wslice(name, k, f0, fn, F):
            return cs[name][:, k * F + f0: k * F + f0 + fn]

        # ================== LN helpers ==================
        def ln_stats(src_tiles, tag):
            """colsum and colsum-of-squares (bf16 src) via ones-matmuls.

            PSUM rows bounce through partition-0 SBUF (engines are
            partition-locked; DMA can't read PSUM), then one SBUF->SBUF
            DMA de-interleaves into [G, L] row layout."""
            st_s = pool.tile([G, L], F32, tag=f"{tag}_s", name=f"{tag}_s", bufs=1)
            st_q = pool.tile([G, L], F32, tag=f"{tag}_q", name=f"{tag}_q", bufs=1)
            bounce = pool.tile([1, G * 2 * L], F32, tag=f"{tag}_bn", name=f"{tag}_bn",
                               bufs=1)
            for sl in range(G):
                ps_sum = ps.tile([128, 512], F32, tag="ps1", name="ps1", bufs=2)
                ps_sq = ps.tile([128, 512], F32, tag="ps1", name="ps1", bufs=2)
                for c0 in range(3):
                    sq = pool.tile([128, L], BF, tag=f"{tag}_sqt", name=f"{tag}_sqt", bufs=3)
                    nc.gpsimd.tensor_tensor(sq[:, :],
                                            src_tiles[c0][:, sl * L:(sl + 1) * L],
                                            src_tiles[c0][:, sl * L:(sl + 1) * L],
                                            OP.mult)
                    nc.tensor.matmul(ps_sum[0:1, 0:L], cs['ones_b'][:, 0:1],
                                     src_tiles[c0][:, sl * L:(sl + 1) * L],
                                     start=(c0 == 0), stop=(c0 == 2))
                    nc.tensor.matmul(ps_sq[0:1, 0:L], cs['ones_b'][:, 0:1], sq[:, :],
                                     start=(c0 == 0), stop=(c0 == 2))
                nc.any.tensor_copy(bounce[:, sl * 2 * L: sl * 2 * L + L],
                                   ps_sum[0:1, 0:L])
                nc.any.tensor_copy(bounce[:, sl * 2 * L + L: (sl + 1) * 2 * L],
                                   ps_sq[0:1, 0:L])
            vb = bounce.rearrange("p (s k i) -> p s k i", s=G, k=2)
            nc.sync.dma_start(st_s.rearrange("s (o i) -> s o i", o=1),
                              vb[:, :, 0:1, :])
            nc.sync.dma_start(st_q.rearrange("s (o i) -> s o i", o=1),
                              vb[:, :, 1:2, :])
            return st_s, st_q

        def ln_rows(st_s, st_q, tag):
            """mean/rstd row-math on [G,L]; returns bf16 [G,L] mean and rstd.

            rstd = sqrt(1/var) via reciprocal_approx_fast (51 ULP) + ACT Sqrt;
            eps dropped (var ~ 1 for these inputs, bf16 noise dominates)."""
            mb = pool.tile([G, L], BF, tag=f"{tag}_mb", name=f"{tag}_mb", bufs=1)
            rb = pool.tile([G, L], BF, tag=f"{tag}_rb", name=f"{tag}_rb", bufs=1)
            t0 = pool.tile([G, L], BF, tag=f"{tag}_t0", name=f"{tag}_t0", bufs=1)
            q = pool.tile([G, L], F32, tag=f"{tag}_qq", name=f"{tag}_qq", bufs=1)
            r = pool.tile([G, L], F32, tag=f"{tag}_r", name=f"{tag}_r", bufs=1)
            nc.vector.tensor_scalar(mb[:], st_s[:], 1.0 / C, None, OP.mult)
            nc.vector.tensor_tensor(t0[:], mb[:], mb[:], OP.mult)
            nc.vector.scalar_tensor_tensor(q[:], st_q[:], 1.0 / C, t0[:],
                                           OP.mult, OP.subtract)   # var
            nc.vector.reciprocal_approx_fast(r[:], q[:])
            nc.scalar.activation(rb[:], r[:], AF.Sqrt)
            return mb, rb

        def ln_apply(src_tiles, mb, rb, tag):
            """xhat = (src - mean)*rstd per chunk/sample -> bf16 tiles."""
            xh = [pool.tile([128, GT], BF, tag=f"{tag}{c0}", name=f"{tag}{c0}", bufs=1)
                  for c0 in range(3)]
            for sl in range(G):
                psm = ps.tile([128, 512], F32, tag="ps1", name="ps1", bufs=2)
                psr = ps.tile([128, 512], F32, tag="ps1", name="ps1", bufs=2)
                nc.tensor.matmul(psm[:, 0:L], cs['e4'][:, sl * 128:(sl + 1) * 128],
                                 mb[:, :])
                nc.tensor.matmul(psr[:, 0:L], cs['e4'][:, sl * 128:(sl + 1) * 128],
                                 rb[:, :])
                # bounce broadcasts to SBUF bf16 so apply ops run in 2x DVE mode
                mbc = pool.tile([128, L], BF, tag=f"{tag}_mbc", name=f"{tag}_mbc", bufs=2)
                rbc = pool.tile([128, L], BF, tag=f"{tag}_rbc", name=f"{tag}_rbc", bufs=2)
                nc.any.tensor_copy(mbc[:, :], psm[:, 0:L])
                nc.any.tensor_copy(rbc[:, :], psr[:, 0:L])
                for c0 in range(3):
                    tmp = pool.tile([128, L], BF, tag=f"{tag}_tmp", name=f"{tag}_tmp", bufs=3)
                    nc.vector.tensor_tensor(tmp[:, :],
                                            src_tiles[c0][:, sl * L:(sl + 1) * L],
                                            mbc[:, :], OP.subtract)
                    nc.vector.tensor_tensor(xh[c0][:, sl * L:(sl + 1) * L],
                                            tmp[:, :], rbc[:, :], OP.mult)
            return xh

        last_exp = [None]  # ACT set-batching: step's gelus wait on its last exp

        # ================== pipeline stage: front(g) ==================
        def front(g):
            X = [pool.tile([128, GT], BF, tag=f"X{c0}", name=f"X{c0}", bufs=2)
                 for c0 in range(3)]
            for sl in range(G):
                for c0 in range(3):
                    nc.sync.dma_start(X[c0][:, sl * L:(sl + 1) * L], xin[g * G + sl, c0])

            st_s, st_q = ln_stats(X, "ln1")
            mb, rb = ln_rows(st_s, st_q, "ln1")
            xh = ln_apply(X, mb, rb, "xh")

            # qk GEMM (feature-major)
            qk = [pool.tile([128, GT], BF, tag=f"qk{f}", name=f"qk{f}", bufs=2)
                  for f in range(6)]
            for f in range(6):
                for sl in range(G):
                    psg = ps.tile([128, 512], F32, tag="ps2", name="ps2", bufs=2)
                    for k in range(3):
                        nc.tensor.matmul(psg[:, 0:L],
                                         wslice('wqk', k, f * 128, 128, 2 * C),
                                         xh[k][:, sl * L:(sl + 1) * L],
                                         start=(k == 0), stop=(k == 2))
                    nc.any.tensor_copy(qk[f][:, sl * L:(sl + 1) * L], psg[:, 0:L])

            # v^T GEMM (token-major, swapped operands)
            # vv[token_part, sl, jc, h, 0:65]: col 64 is the ones column.
            vT = pool.tile([128, G * 3 * (NH * NBLK)], BF, tag="vT", name="vT", bufs=2)
            vv = vT.rearrange("p (s t h c) -> p s t h c", s=G, t=3, h=NH)
            for sl in range(G):
                for jc, (j0, jw) in enumerate(JCH):
                    psv = ps.tile([128, 512], F32, tag="ps1", name="ps1", bufs=2)
                    for k in range(3):
                        nc.tensor.matmul(psv[0:jw, 0:C],
                                         xh[k][:, sl * L + j0: sl * L + j0 + jw],
                                         wslice('wv', k, 0, C, C),
                                         start=(k == 0), stop=(k == 2))
                    pv = psv[:, 0:C].rearrange("p (h c) -> p h c", h=NH)[0:jw, :, 0:64]
                    nc.any.tensor_copy(vv[0:jw, sl, jc, :, 0:64], pv)
                    nc.gpsimd.memset(vv[0:jw, sl, jc, :, 64:65], 1.0)
            return dict(X=X, qk=qk, vv=vv)

        # ================== pipeline stage: mid(g) ==================
        def mid(g, st):
            X, qk, vv = st['X'], st['qk'], st['vv']
            # attention per (sample, head-pair): head pair (2hp, 2hp+1) lives on
            # partitions 0:64 / 64:128 of the same qk tile -> the two S matmuls
            # row-pack onto PE row groups.
            O = [pool.tile([128, GT], BF, tag=f"O{c0}", name=f"O{c0}", bufs=1)
                 for c0 in range(3)]
            for sl in range(G):
                rinv_raw = pool.tile([NH, L], F32, tag="rinv_raw", name="rinv_raw", bufs=2)
                o_un = pool.tile([128, 3 * L], BF, tag="o_un", name="o_un", bufs=2)
                dbs = []
                for hp in range(3):
                    PT = pool.tile([128, 2 * 3 * L], BF, tag="PT", name="PT", bufs=2)
                    vPT = PT.rearrange("p (h j i) -> p h j i", h=2, j=3)
                    PT2 = pool.tile([128, 2 * 3 * L], BF, tag="PT2", name="PT2", bufs=2)
                    vPT2 = PT2.rearrange("p (h j i) -> p h j i", h=2, j=3)
                    veb = cs['eb'].rearrange("p (j h i) -> p j h i", j=3, h=NH)
                    for jc, (j0, jw) in enumerate(JCH):
                        psS = ps.tile([128, 1024], F32, tag="psS", name="psS", bufs=2)
                        for ph in range(2):
                            pq = ph * 64
                            nc.tensor.matmul(
                                psS[0:jw, ph * 512: ph * 512 + L],
                                qk[3 + hp][pq:pq + 64, sl * L + j0: sl * L + j0 + jw],
                                qk[hp][pq:pq + 64, sl * L:(sl + 1) * L])
                        vS = psS.rearrange("p (h c) -> p h c", h=2)[0:jw, :, 0:L]
                        last_exp[0] = nc.scalar.activation(vPT[0:jw, :, jc, :], vS,
                                                           AF.Exp)
                        # per-jc bias multiply so PV can start before later exps
                        nc.vector.tensor_tensor(vPT2[0:jw, :, jc, :], vPT[0:jw, :, jc, :],
                                                veb[0:jw, jc, hp * 2:hp * 2 + 2, :],
                                                OP.mult)
                    psO = ps.tile([128, 1024], F32, tag="psS", name="psS", bufs=2)
                    for ph in range(2):
                        h = hp * 2 + ph
                        for jc, (j0, jw) in enumerate(JCH):
                            nc.tensor.matmul(psO[0:NBLK, ph * 512: ph * 512 + L],
                                             vv[0:jw, sl, jc, h, :],
                                             vPT2[0:jw, ph, jc, :],
                                             start=(jc == 0), stop=(jc == 2))
                    # unnormalized O: even head stays on p0:64; odd head bounces
                    # through SBUF + DMA to land on p64:128 of the pair block.
                    nc.any.tensor_copy(o_un[0:64, hp * L:(hp + 1) * L], psO[0:64, 0:L])
                    scr = pool.tile([64, L], BF, tag="oscr", name="oscr", bufs=3)
                    nc.any.tensor_copy(scr[:, :], psO[0:64, 512:512 + L])
                    nc.sync.dma_start(o_un[64:128, hp * L:(hp + 1) * L], scr[:, :])
                    # denominator rows for both heads in one strided copy
                    db = pool.tile([128, 2 * L], F32, tag="dband", name="dband", bufs=3)
                    vdb = db[64:65, :].rearrange("p (h i) -> p h i", h=2)
                    vpsO = psO.rearrange("p (h c) -> p h c", h=2)
                    nc.any.tensor_copy(vdb, vpsO[64:65, :, 0:L])
                    dbs.append(db)
                for hp in range(3):
                    nc.sync.dma_start(
                        rinv_raw[2 * hp:2 * hp + 2, :].rearrange("h (o i) -> h o i", o=1),
                        dbs[hp][64:65, :].rearrange("p (h i) -> p h i", h=2))
                # rinv = 1/(denom + PB)
                nc.vector.tensor_tensor(rinv_raw[:], rinv_raw[:], cs['pb'][:, :], OP.add)
                rinv_f = pool.tile([NH, L], F32, tag="rinv_f", name="rinv_f", bufs=2)
                nc.vector.reciprocal_approx_fast(rinv_f[:], rinv_raw[:])
                rinv_b = pool.tile([NH, L], BF, tag="rinv_b", name="rinv_b", bufs=2)
                nc.vector.tensor_copy(rinv_b[:], rinv_f[:])
                for m9 in range(3):
                    psR = ps.tile([128, 512], F32, tag="ps1", name="ps1", bufs=2)
                    nc.tensor.matmul(psR[:, 0:L], cs['bsel'][:, m9 * 128:(m9 + 1) * 128],
                                     rinv_b[:, :])
                    nc.vector.tensor_tensor(O[m9][:, sl * L:(sl + 1) * L],
                                            o_un[:, m9 * L:(m9 + 1) * L],
                                            psR[:, 0:L], OP.mult)

            # proj + shortcut
            t1 = [pool.tile([128, GT], BF, tag=f"t1_{f}", name=f"t1_{f}", bufs=2)
                  for f in range(3)]
            for f in range(3):
                for sl in range(G):
                    psg = ps.tile([128, 512], F32, tag="ps2", name="ps2", bufs=2)
                    for k in range(3):
                        nc.tensor.matmul(psg[:, 0:L],
                                         wslice('wp', k, f * 128, 128, C),
                                         O[k][:, sl * L:(sl + 1) * L],
                                         start=(k == 0), stop=(k == 2))
                    nc.vector.tensor_tensor(t1[f][:, sl * L:(sl + 1) * L], psg[:, 0:L],
                                            X[f][:, sl * L:(sl + 1) * L], OP.add)
            st['t1'] = t1

        # ================== pipeline stage: tail(g) ==================
        def tail(g, st):
            t1 = st['t1']
            st2_s, st2_q = ln_stats(t1, "ln2")
            mb2, rb2 = ln_rows(st2_s, st2_q, "ln2")
            xh2 = ln_apply(t1, mb2, rb2, "xh2")

            fc1h = [pool.tile([128, GT], BF, tag=f"fc1h{f}", name=f"fc1h{f}", bufs=2)
                    for f in range(12)]
            for f in range(12):
                for sl in range(G):
                    psg = ps.tile([128, 512], F32, tag="ps2", name="ps2", bufs=2)
                    for k in range(3):
                        nc.tensor.matmul(psg[:, 0:L],
                                         wslice('w1', k, f * 128, 128, 4 * C),
                                         xh2[k][:, sl * L:(sl + 1) * L],
                                         start=(k == 0), stop=(k == 2))
                    nc.scalar.activation(fc1h[f][:, sl * L:(sl + 1) * L],
                                         psg[:, 0:L], AF.Gelu)

            t2 = [pool.tile([128, GT], BF, tag=f"t2_{f}", name=f"t2_{f}", bufs=1)
                  for f in range(3)]
            for f in range(3):
                for sl in range(G):
                    psg = ps.tile([128, 512], F32, tag="ps2", name="ps2", bufs=2)
                    for k in range(12):
                        nc.tensor.matmul(psg[:, 0:L],
                                         wslice('w2', k, f * 128, 128, C),
                                         fc1h[k][:, sl * L:(sl + 1) * L],
                                         start=(k == 0), stop=(k == 11))
                    nc.vector.tensor_tensor(t2[f][:, sl * L:(sl + 1) * L], psg[:, 0:L],
                                            t1[f][:, sl * L:(sl + 1) * L], OP.add)

            # output GEMM + relu
            t_out = [pool.tile([128, GT], BF, tag=f"to{f}", name=f"to{f}", bufs=1)
                     for f in range(2)]
            for f in range(2):
                fw = 128 if f == 0 else 64
                for sl in range(G):
                    psg = ps.tile([128, 512], F32, tag="ps2", name="ps2", bufs=2)
                    for k in range(3):
                        nc.tensor.matmul(psg[0:fw, 0:L],
                                         wslice('wo', k, f * 128, fw, OUT_DIM),
                                         t2[k][:, sl * L:(sl + 1) * L],
                                         start=(k == 0), stop=(k == 2))
                    nc.scalar.activation(t_out[f][0:fw, sl * L:(sl + 1) * L],
                                         psg[0:fw, 0:L], AF.Relu)

            # bilinear 2x upsample: premultiplies on DVE/ACT, shifted adds on
            # gpsimd tensor_tensor (its tensor_scalar ucode is pathologically
            # slow), edge copies on DVE.
            for sl in range(G):
                for f in range(2):
                    PC = 128 if f == 0 else 64
                    tin = t_out[f][0:PC, sl * L:(sl + 1) * L]
                    p25 = pool.tile([128, L], BF, tag="p25", name="p25", bufs=2)
                    p75 = pool.tile([128, L], BF, tag="p75", name="p75", bufs=2)
                    nc.scalar.activation(p25[0:PC, :], tin, AF.Copy, scale=0.25)
                    nc.vector.tensor_scalar(p75[0:PC, :], tin, 0.75, None, OP.mult)
                    XI = pool.tile([128, 2 * L], BF, tag="XI", name="XI", bufs=2)
                    vXI = XI.rearrange("p (y x t) -> p y x t", y=15, x=20)
                    v25 = p25.rearrange("p (y x o) -> p y x o", y=15, x=20)
                    v75 = p75.rearrange("p (y x o) -> p y x o", y=15, x=20)
                    vti = tin.rearrange("p (y x o) -> p y x o", y=15, x=20)
                    # even cols: out[2x] = .25 in[x-1] + .75 in[x]; edge x=0 copy
                    nc.gpsimd.tensor_tensor(vXI[0:PC, :, 1:20, 0:1], v25[0:PC, :, 0:19, :],
                                            v75[0:PC, :, 1:20, :], OP.add)
                    nc.vector.tensor_copy(vXI[0:PC, :, 0:1, 0:1], vti[0:PC, :, 0:1, :])
                    # odd cols: out[2x+1] = .75 in[x] + .25 in[x+1]; edge x=19 copy
                    nc.gpsimd.tensor_tensor(vXI[0:PC, :, 0:19, 1:2], v75[0:PC, :, 0:19, :],
                                            v25[0:PC, :, 1:20, :], OP.add)
                    nc.vector.tensor_copy(vXI[0:PC, :, 19:20, 1:2], vti[0:PC, :, 19:20, :])
                    # y-pass: EY[y] = .25 XI[y-1] + .75 XI[y]; OY[y] = .75 XI[y] + .25 XI[y+1]
                    p25y = pool.tile([128, 2 * L], BF, tag="p25y", name="p25y", bufs=2)
                    p75y = pool.tile([128, 2 * L], BF, tag="p75y", name="p75y", bufs=2)
                    nc.scalar.activation(p25y[0:PC, :], XI[0:PC, :], AF.Copy, scale=0.25)
                    nc.vector.tensor_scalar(p75y[0:PC, :], XI[0:PC, :], 0.75, None, OP.mult)
                    EY = pool.tile([128, 2 * L], BF, tag="EY", name="EY", bufs=2)
                    OY = pool.tile([128, 2 * L], BF, tag="OY", name="OY", bufs=2)
                    vEY = EY.rearrange("p (y x) -> p y x", y=15)
                    vOY = OY.rearrange("p (y x) -> p y x", y=15)
                    vq25 = p25y.rearrange("p (y x) -> p y x", y=15)
                    vq75 = p75y.rearrange("p (y x) -> p y x", y=15)
                    vXI2 = XI.rearrange("p (y x) -> p y x", y=15)
                    nc.gpsimd.tensor_tensor(vEY[0:PC, 1:15, :], vq25[0:PC, 0:14, :],
                                            vq75[0:PC, 1:15, :], OP.add)
                    nc.vector.tensor_copy(vEY[0:PC, 0:1, :], vXI2[0:PC, 0:1, :])
                    nc.gpsimd.tensor_tensor(vOY[0:PC, 0:14, :], vq75[0:PC, 0:14, :],
                                            vq25[0:PC, 1:15, :], OP.add)
                    nc.vector.tensor_copy(vOY[0:PC, 14:15, :], vXI2[0:PC, 14:15, :])
                    ov = outd[g * G + sl, f * 128:f * 128 + PC].rearrange(
                        "c (y t) x -> c y (t x)", t=2)
                    nc.sync.dma_start(ov[:, :, 0:40], vEY[0:PC, :, :])
                    nc.sync.dma_start(ov[:, :, 40:80], vOY[0:PC, :, :])

        # ================== run the 3-stage pipeline ==================
        n_groups = n_samples // G
        states = {}
        for gg in range(n_groups + 2):
            if gg < n_groups:
                states[gg] = front(gg)
            if 1 <= gg < n_groups + 1:
                mid(gg - 1, states[gg - 1])
            if gg >= 2:
                tail(gg - 2, states[gg - 2])
                del states[gg - 2]

    nc.compile()
    return nc


_PROG_CACHE = {}


def kernel(x, ln1_s, ln1_b, qkv_w, qkv_b, bias_table, proj_w, proj_b,
           ln2_s, ln2_b, fc1_w, fc1_b, fc2_w, fc2_b, out_w, out_b):
    from concourse.bass_utils import run_bass_kernel_spmd
    x = np.asarray(x, np.float32)
    consts = build_consts(ln1_s, ln1_b, qkv_w, qkv_b, bias_table, proj_w, proj_b,
                          ln2_s, ln2_b, fc1_w, fc1_b, fc2_w, fc2_b, out_w, out_b)
    if S not in _PROG_CACHE:
        _PROG_CACHE[S] = build_program(S)
    nc = _PROG_CACHE[S]
    xs = x.reshape(B, 3, 128, H * W).astype(bf16)
    in_maps = []
    for cid in range(N_CORES):
        m = {'xin': np.ascontiguousarray(xs[cid * S:(cid + 1) * S])}
        m.update(consts)
        in_maps.append(m)
    res = run_bass_kernel_spmd(nc, in_maps, core_ids=list(range(N_CORES)))
    out = np.concatenate([r['out'] for r in res.results], axis=0)
    return out.astype(np.float32)


# revision 17
# speedup vs baseline: 1.0189x; 1.0189x over previous
"""Trainium2 Bass kernel for AttnDecoderBlock (window attention + MLP + bilinear upsample).

Strategy: pure data-parallel over batch B=128 -> 8 cores x 16 samples.
On-chip layout is feature-major [C_partition, token_free].  LN scale/bias are
folded into the following GEMM weights on the host; attention uses the
S^T = k^T q orientation with exp(S)*exp(bias) and a host-precomputed
padded-key denominator correction PB[h,i] = sum_{j>=300} exp(bias[h,i,j]).
All GEMMs bf16 with fp32 PSUM accumulation; residual trunk bf16.

v6: three-stage software pipeline across groups -- front(g) = load+LN1+qk+v,
mid(g-1) = attention+proj, tail(g-2) = LN2+MLP+out+upsample -- so the PE has
ready GEMM work from adjacent groups during each group's serial LN/softmax
chains.  LN rstd via all-DVE Newton rsqrt (keeps Sqrt off the ACT engine --
table-set switches between Exp/Gelu/Sqrt cost 2.7us each).
"""

import numpy as np
import ml_dtypes
from contextlib import ExitStack

from concourse import bacc, mybir
from concourse.bass import _add_dep_helper
import concourse.bass as bass
import concourse.tile as tile

dt = mybir.dt
BF = dt.bfloat16
F32 = dt.float32
AF = mybir.ActivationFunctionType
OP = mybir.AluOpType

# problem constants (hardcoded per spec)
B, C, NH, WS, H, W = 128, 384, 6, 20, 15, 20
HD = C // NH            # 64
L = H * W               # 300 real tokens
N = WS * WS             # 400 padded tokens
OUT_DIM, OUT_H, OUT_W = 192, 30, 40
N_CORES = 8
S = B // N_CORES        # 16 samples per core
G = 2                   # samples per group (token-batch for GEMM stages)
GT = G * L              # 600
NBLK = 65               # v^T block width per head: 64 dims + 1 ones col
JCH = [(0, 128), (128, 128), (256, 44)]   # attention key/token chunks
bf16 = ml_dtypes.bfloat16


def _rel_pos_index(ws):
    coords = np.stack(np.meshgrid(np.arange(ws), np.arange(ws), indexing='ij')).reshape(2, -1)
    rel = (coords[:, :, None] - coords[:, None, :]).transpose(1, 2, 0)
    rel[:, :, 0] += ws - 1
    rel[:, :, 1] += ws - 1
    rel[:, :, 0] *= 2 * ws - 1
    return rel.sum(-1)


def build_consts(ln1_s, ln1_b, qkv_w, qkv_b, bias_table, proj_w, proj_b,
                 ln2_s, ln2_b, fc1_w, fc1_b, fc2_w, fc2_b, out_w, out_b):
    """Host-side weight folding and layout. Returns dict name -> np array."""
    f32 = np.float32
    qkv_w = np.asarray(qkv_w, f32); qkv_b = np.asarray(qkv_b, f32)
    ln1_s = np.asarray(ln1_s, f32); ln1_b = np.asarray(ln1_b, f32)
    # fold LN1 affine into qkv weights; fold attention scale into q
    Wq = ln1_s[:, None] * qkv_w[:, 0:C] * (HD ** -0.5)
    Wk = ln1_s[:, None] * qkv_w[:, C:2 * C]
    Wv = ln1_s[:, None] * qkv_w[:, 2 * C:3 * C]
    bq = (ln1_b @ qkv_w[:, 0:C] + qkv_b[0:C]) * (HD ** -0.5)
    bk = ln1_b @ qkv_w[:, C:2 * C] + qkv_b[C:2 * C]
    bv = ln1_b @ qkv_w[:, 2 * C:] + qkv_b[2 * C:]
    assert not np.any(bq) and not np.any(bk) and not np.any(bv), \
        "nonzero qkv/ln1 bias path not implemented"
    Wqk = np.concatenate([Wq, Wk], axis=1)            # [C, 768]
    W1 = np.asarray(ln2_s, f32)[:, None] * np.asarray(fc1_w, f32)
    b1 = np.asarray(ln2_b, f32) @ np.asarray(fc1_w, f32) + np.asarray(fc1_b, f32)
    assert not np.any(b1) and not np.any(proj_b) and not np.any(fc2_b) and not np.any(out_b), \
        "nonzero bias path not implemented"

    REL = _rel_pos_index(WS)
    bias = np.asarray(bias_table, f32)[REL].transpose(2, 0, 1)   # [NH, 400, 400]
    EB_T = np.exp(bias[:, :L, :L].transpose(0, 2, 1))            # [NH, j, i]
    PB = np.exp(bias[:, :L, L:]).sum(-1)                         # [NH, 300]

    bsel = np.zeros((NH, 3 * 128), f32)
    for h in range(NH):
        bsel[h, (h // 2) * 128 + (h % 2) * 64: (h // 2) * 128 + (h % 2) * 64 + 64] = 1.0
    e4 = np.zeros((G, G * 128), f32)
    for s in range(G):
        e4[s, s * 128:(s + 1) * 128] = 1.0

    c = {}
    c['wqk'] = np.ascontiguousarray(Wqk.reshape(3, 128, 2 * C)).astype(bf16)
    c['wv'] = np.ascontiguousarray(Wv.reshape(3, 128, C)).astype(bf16)
    c['wp'] = np.ascontiguousarray(np.asarray(proj_w, f32).reshape(3, 128, C)).astype(bf16)
    c['w1'] = np.ascontiguousarray(W1.reshape(3, 128, 4 * C)).astype(bf16)
    c['w2'] = np.ascontiguousarray(np.asarray(fc2_w, f32).reshape(12, 128, C)).astype(bf16)
    c['wo'] = np.ascontiguousarray(np.asarray(out_w, f32).reshape(3, 128, OUT_DIM)).astype(bf16)
    # EB^T in j-chunk layout: [jc, NH, jw<=128 partitions, 300]
    ebt = np.zeros((3, NH, 128, L), f32)
    for jc, (j0, jw) in enumerate(JCH):
        ebt[jc, :, 0:jw, :] = EB_T[:, j0:j0 + jw, :]
    c['eb'] = ebt.astype(bf16)                                   # [3, NH, 128, 300]
    c['pb'] = PB.astype(f32)                                     # [6, 300]
    c['bsel'] = bsel.astype(bf16)                                # [6, 384]
    c['e4'] = e4.astype(bf16)                                    # [G, G*128]
    c['ones_b'] = np.ones((128, 1), bf16)
    return c


CONST_SPECS = [
    ('wqk', (3, 128, 2 * C), BF), ('wv', (3, 128, C), BF), ('wp', (3, 128, C), BF),
    ('w1', (3, 128, 4 * C), BF), ('w2', (12, 128, C), BF), ('wo', (3, 128, OUT_DIM), BF),
    ('eb', (3, NH, 128, L), BF), ('pb', (NH, L), F32),
    ('bsel', (NH, 3 * 128), BF), ('e4', (G, G * 128), BF),
    ('ones_b', (128, 1), BF),
]


def build_program(n_samples, debug=False):
    """Build the Bass program for one core processing n_samples samples."""
    nc = bacc.Bacc(None, target_bir_lowering=False, debug=debug)
    xin = nc.dram_tensor("xin", [n_samples, 3, 128, L], BF, kind="ExternalInput")
    outd = nc.dram_tensor("out", [n_samples, OUT_DIM, OUT_H, OUT_W], BF,
                          kind="ExternalOutput")
    cdram = {name: nc.dram_tensor(name, list(shape), d, kind="ExternalInput")
             for name, shape, d in CONST_SPECS}

    with tile.TileContext(nc) as tc, ExitStack() as ctx:
        cpool = ctx.enter_context(tc.tile_pool(name="consts", bufs=1))
        pool = ctx.enter_context(tc.tile_pool(name="main", bufs=1))
        ps = ctx.enter_context(tc.tile_pool(name="psum", bufs=1, space="PSUM"))

        # ---- resident constants -> SBUF
        cs = {}
        for name, shape, d in CONST_SPECS:
            if len(shape) == 2:
                t = cpool.tile([shape[0] if shape[0] > 1 else 1, shape[1]], d, tag=name, name=name)
                nc.sync.dma_start(t[:], cdram[name][:])
            elif name == 'eb':
                t = cpool.tile([128, 3 * NH * L], d, tag=name, name=name)
                nc.sync.dma_start(t.rearrange("p (j h i) -> p j h i", j=3, h=NH),
                                  cdram[name].rearrange("j h p i -> p j h i"))
            else:  # [k, 128, F] weight stacks
                k, p, f = shape
                t = cpool.tile([128, k * f], d, tag=name, name=name)
                nc.sync.dma_start(t.rearrange("p (k f) -> p k f", k=k),
                                  cdram[name].rearrange("k p f -> p k f"))
            cs[name] = t

        def wslice(name, k, f0, fn, F):
            return cs[name][:, k * F + f0: k * F + f0 + fn]

        # ================== LN helpers ==================
        def ln_stats(src_tiles, tag):
            """colsum and colsum-of-squares (bf16 src) via ones-matmuls.

            PSUM rows bounce through partition-0 SBUF (engines are
            partition-locked; DMA can't read PSUM), then one SBUF->SBUF
            DMA de-interleaves into [G, L] row layout."""
            st_s = pool.tile([G, L], F32, tag=f"{tag}_s", name=f"{tag}_s", bufs=1)
            st_q = pool.tile([G, L], F32, tag=f"{tag}_q", name=f"{tag}_q", bufs=1)
            bounce = pool.tile([1, G * 2 * L], F32, tag=f"{tag}_bn", name=f"{tag}_bn",
                               bufs=1)
            for sl in range(G):
                ps_sum = ps.tile([128, 512], F32, tag="ps1", name="ps1", bufs=2)
                ps_sq = ps.tile([128, 512], F32, tag="ps1", name="ps1", bufs=2)
                for c0 in range(3):
                    sq = pool.tile([128, L], BF, tag=f"{tag}_sqt", name=f"{tag}_sqt", bufs=3)
                    nc.gpsimd.tensor_tensor(sq[:, :],
                                            src_tiles[c0][:, sl * L:(sl + 1) * L],
                                            src_tiles[c0][:, sl * L:(sl + 1) * L],
                                            OP.mult)
                    nc.tensor.matmul(ps_sum[0:1, 0:L], cs['ones_b'][:, 0:1],
                                     src_tiles[c0][:, sl * L:(sl + 1) * L],
                                     start=(c0 == 0), stop=(c0 == 2))
                    nc.tensor.matmul(ps_sq[0:1, 0:L], cs['ones_b'][:, 0:1], sq[:, :],
                                     start=(c0 == 0), stop=(c0 == 2))
                nc.any.tensor_copy(bounce[:, sl * 2 * L: sl * 2 * L + L],
                                   ps_sum[0:1, 0:L])
                nc.any.tensor_copy(bounce[:, sl * 2 * L + L: (sl + 1) * 2 * L],
                                   ps_sq[0:1, 0:L])
            vb = bounce.rearrange("p (s k i) -> p s k i", s=G, k=2)
            nc.sync.dma_start(st_s.rearrange("s (o i) -> s o i", o=1),
                              vb[:, :, 0:1, :])
            nc.sync.dma_start(st_q.rearrange("s (o i) -> s o i", o=1),
                              vb[:, :, 1:2, :])
            return st_s, st_q

        def ln_rows(st_s, st_q, tag):
            """mean/rstd row-math on [G,L]; returns bf16 [G,L] mean and rstd.

            rstd = sqrt(1/var) via reciprocal_approx_fast (51 ULP) + ACT Sqrt;
            eps dropped (var ~ 1 for these inputs, bf16 noise dominates)."""
            mb = pool.tile([G, L], BF, tag=f"{tag}_mb", name=f"{tag}_mb", bufs=1)
            rb = pool.tile([G, L], BF, tag=f"{tag}_rb", name=f"{tag}_rb", bufs=1)
            t0 = pool.tile([G, L], BF, tag=f"{tag}_t0", name=f"{tag}_t0", bufs=1)
            q = pool.tile([G, L], F32, tag=f"{tag}_qq", name=f"{tag}_qq", bufs=1)
            r = pool.tile([G, L], F32, tag=f"{tag}_r", name=f"{tag}_r", bufs=1)
            nc.vector.tensor_scalar(mb[:], st_s[:], 1.0 / C, None, OP.mult)
            nc.vector.tensor_tensor(t0[:], mb[:], mb[:], OP.mult)
            nc.vector.scalar_tensor_tensor(q[:], st_q[:], 1.0 / C, t0[:],
                                           OP.mult, OP.subtract)   # var
            nc.vector.reciprocal_approx_fast(r[:], q[:])
            nc.scalar.activation(rb[:], r[:], AF.Sqrt)
            return mb, rb

        def ln_apply(src_tiles, mb, rb, tag):
            """xhat = (src - mean)*rstd per chunk/sample -> bf16 tiles."""
            xh = [pool.tile([128, GT], BF, tag=f"{tag}{c0}", name=f"{tag}{c0}", bufs=1)
                  for c0 in range(3)]
            for sl in range(G):
                psm = ps.tile([128, 512], F32, tag="ps1", name="ps1", bufs=2)
                psr = ps.tile([128, 512], F32, tag="ps1", name="ps1", bufs=2)
                nc.tensor.matmul(psm[:, 0:L], cs['e4'][:, sl * 128:(sl + 1) * 128],
                                 mb[:, :])
                nc.tensor.matmul(psr[:, 0:L], cs['e4'][:, sl * 128:(sl + 1) * 128],
                                 rb[:, :])
                # bounce broadcasts to SBUF bf16 so apply ops run in 2x DVE mode
                mbc = pool.tile([128, L], BF, tag=f"{tag}_mbc", name=f"{tag}_mbc", bufs=2)
                rbc = pool.tile([128, L], BF, tag=f"{tag}_rbc", name=f"{tag}_rbc", bufs=2)
                nc.any.tensor_copy(mbc[:, :], psm[:, 0:L])
                nc.any.tensor_copy(rbc[:, :], psr[:, 0:L])
                for c0 in range(3):
                    tmp = pool.tile([128, L], BF, tag=f"{tag}_tmp", name=f"{tag}_tmp", bufs=3)
                    nc.vector.tensor_tensor(tmp[:, :],
                                            src_tiles[c0][:, sl * L:(sl + 1) * L],
                                            mbc[:, :], OP.subtract)
                    nc.vector.tensor_tensor(xh[c0][:, sl * L:(sl + 1) * L],
                                            tmp[:, :], rbc[:, :], OP.mult)
            return xh

        last_exp = [None]  # ACT set-batching: step's gelus wait on its last exp

        # ================== pipeline stage: front(g) ==================
        def front(g):
            X = [pool.tile([128, GT], BF, tag=f"X{c0}", name=f"X{c0}", bufs=3)
                 for c0 in range(3)]
            for sl in range(G):
                for c0 in range(3):
                    nc.sync.dma_start(X[c0][:, sl * L:(sl + 1) * L], xin[g * G + sl, c0])

            st_s, st_q = ln_stats(X, "ln1")
            mb, rb = ln_rows(st_s, st_q, "ln1")
            xh = ln_apply(X, mb, rb, "xh")

            # qk GEMM (feature-major)
            qk = [pool.tile([128, GT], BF, tag=f"qk{f}", name=f"qk{f}", bufs=3)
                  for f in range(6)]
            for f in range(6):
                for sl in range(G):
                    psg = ps.tile([128, 512], F32, tag="ps2", name="ps2", bufs=2)
                    for k in range(3):
                        nc.tensor.matmul(psg[:, 0:L],
                                         wslice('wqk', k, f * 128, 128, 2 * C),
                                         xh[k][:, sl * L:(sl + 1) * L],
                                         start=(k == 0), stop=(k == 2))
                    nc.any.tensor_copy(qk[f][:, sl * L:(sl + 1) * L], psg[:, 0:L])

            # v^T GEMM (token-major, swapped operands)
            # vv[token_part, sl, jc, h, 0:65]: col 64 is the ones column.
            vT = pool.tile([128, G * 3 * (NH * NBLK)], BF, tag="vT", name="vT", bufs=3)
            vv = vT.rearrange("p (s t h c) -> p s t h c", s=G, t=3, h=NH)
            for sl in range(G):
                for jc, (j0, jw) in enumerate(JCH):
                    psv = ps.tile([128, 512], F32, tag="ps1", name="ps1", bufs=2)
                    for k in range(3):
                        nc.tensor.matmul(psv[0:jw, 0:C],
                                         xh[k][:, sl * L + j0: sl * L + j0 + jw],
                                         wslice('wv', k, 0, C, C),
                                         start=(k == 0), stop=(k == 2))
                    pv = psv[:, 0:C].rearrange("p (h c) -> p h c", h=NH)[0:jw, :, 0:64]
                    nc.any.tensor_copy(vv[0:jw, sl, jc, :, 0:64], pv)
                    nc.gpsimd.memset(vv[0:jw, sl, jc, :, 64:65], 1.0)
            return dict(X=X, qk=qk, vv=vv)

        # ================== pipeline stage: mid(g) ==================
        def mid(g, st):
            X, qk, vv = st['X'], st['qk'], st['vv']
            # attention per (sample, head-pair): head pair (2hp, 2hp+1) lives on
            # partitions 0:64 / 64:128 of the same qk tile -> the two S matmuls
            # row-pack onto PE row groups.
            O = [pool.tile([128, GT], BF, tag=f"O{c0}", name=f"O{c0}", bufs=1)
                 for c0 in range(3)]
            for sl in range(G):
                rinv_raw = pool.tile([NH, L], F32, tag="rinv_raw", name="rinv_raw", bufs=2)
                o_un = pool.tile([128, 3 * L], BF, tag="o_un", name="o_un", bufs=2)
                dbs = []
                for hp in range(3):
                    PT = pool.tile([128, 2 * 3 * L], BF, tag="PT", name="PT", bufs=2)
                    vPT = PT.rearrange("p (h j i) -> p h j i", h=2, j=3)
                    PT2 = pool.tile([128, 2 * 3 * L], BF, tag="PT2", name="PT2", bufs=2)
                    vPT2 = PT2.rearrange("p (h j i) -> p h j i", h=2, j=3)
                    veb = cs['eb'].rearrange("p (j h i) -> p j h i", j=3, h=NH)
                    for jc, (j0, jw) in enumerate(JCH):
                        psS = ps.tile([128, 1024], F32, tag="psS", name="psS", bufs=2)
                        for ph in range(2):
                            pq = ph * 64
                            nc.tensor.matmul(
                                psS[0:jw, ph * 512: ph * 512 + L],
                                qk[3 + hp][pq:pq + 64, sl * L + j0: sl * L + j0 + jw],
                                qk[hp][pq:pq + 64, sl * L:(sl + 1) * L])
                        vS = psS.rearrange("p (h c) -> p h c", h=2)[0:jw, :, 0:L]
                        last_exp[0] = nc.scalar.activation(vPT[0:jw, :, jc, :], vS,
                                                           AF.Exp)
                        # per-jc bias multiply so PV can start before later exps
                        nc.vector.tensor_tensor(vPT2[0:jw, :, jc, :], vPT[0:jw, :, jc, :],
                                                veb[0:jw, jc, hp * 2:hp * 2 + 2, :],
                                                OP.mult)
                    psO = ps.tile([128, 1024], F32, tag="psS", name="psS", bufs=2)
                    for ph in range(2):
                        h = hp * 2 + ph
                        for jc, (j0, jw) in enumerate(JCH):
                            nc.tensor.matmul(psO[0:NBLK, ph * 512: ph * 512 + L],
                                             vv[0:jw, sl, jc, h, :],
                                             vPT2[0:jw, ph, jc, :],
                                             start=(jc == 0), stop=(jc == 2))
                    # unnormalized O: even head stays on p0:64; odd head bounces
                    # through SBUF + DMA to land on p64:128 of the pair block.
                    nc.any.tensor_copy(o_un[0:64, hp * L:(hp + 1) * L], psO[0:64, 0:L])
                    scr = pool.tile([64, L], BF, tag="oscr", name="oscr", bufs=3)
                    nc.any.tensor_copy(scr[:, :], psO[0:64, 512:512 + L])
                    nc.sync.dma_start(o_un[64:128, hp * L:(hp + 1) * L], scr[:, :])
                    # denominator rows for both heads in one strided copy
                    db = pool.tile([128, 2 * L], F32, tag="dband", name="dband", bufs=3)
                    vdb = db[64:65, :].rearrange("p (h i) -> p h i", h=2)
                    vpsO = psO.rearrange("p (h c) -> p h c", h=2)
                    nc.any.tensor_copy(vdb, vpsO[64:65, :, 0:L])
                    dbs.append(db)
                for hp in range(3):
                    nc.sync.dma_start(
                        rinv_raw[2 * hp:2 * hp + 2, :].rearrange("h (o i) -> h o i", o=1),
                        dbs[hp][64:65, :].rearrange("p (h i) -> p h i", h=2))
                # rinv = 1/(denom + PB)
                nc.vector.tensor_tensor(rinv_raw[:], rinv_raw[:], cs['pb'][:, :], OP.add)
                rinv_f = pool.tile([NH, L], F32, tag="rinv_f", name="rinv_f", bufs=2)
                nc.vector.reciprocal_approx_fast(rinv_f[:], rinv_raw[:])
                rinv_b = pool.tile([NH, L], BF, tag="rinv_b", name="rinv_b", bufs=2)
                nc.vector.tensor_copy(rinv_b[:], rinv_f[:])
                for m9 in range(3):
                    psR = ps.tile([128, 512], F32, tag="ps1", name="ps1", bufs=2)
                    nc.tensor.matmul(psR[:, 0:L], cs['bsel'][:, m9 * 128:(m9 + 1) * 128],
                                     rinv_b[:, :])
                    nc.vector.tensor_tensor(O[m9][:, sl * L:(sl + 1) * L],
                                            o_un[:, m9 * L:(m9 + 1) * L],
                                            psR[:, 0:L], OP.mult)

            # proj + shortcut
            t1 = [pool.tile([128, GT], BF, tag=f"t1_{f}", name=f"t1_{f}", bufs=2)
                  for f in range(3)]
            for f in range(3):
                for sl in range(G):
                    psg = ps.tile([128, 512], F32, tag="ps2", name="ps2", bufs=2)
                    for k in range(3):
                        nc.tensor.matmul(psg[:, 0:L],
                                         wslice('wp', k, f * 128, 128, C),
                                         O[k][:, sl * L:(sl + 1) * L],
                                         start=(k == 0), stop=(k == 2))
                    nc.vector.tensor_tensor(t1[f][:, sl * L:(sl + 1) * L], psg[:, 0:L],
                                            X[f][:, sl * L:(sl + 1) * L], OP.add)
            st['t1'] = t1

        # ================== pipeline stage: tail(g) ==================
        def tail(g, st):
            t1 = st['t1']
            st2_s, st2_q = ln_stats(t1, "ln2")
            mb2, rb2 = ln_rows(st2_s, st2_q, "ln2")
            xh2 = ln_apply(t1, mb2, rb2, "xh2")

            fc1h = [pool.tile([128, GT], BF, tag=f"fc1h{f}", name=f"fc1h{f}", bufs=1)
                    for f in range(12)]
            for f in range(12):
                for sl in range(G):
                    psg = ps.tile([128, 512], F32, tag="ps2", name="ps2", bufs=2)
                    for k in range(3):
                        nc.tensor.matmul(psg[:, 0:L],
                                         wslice('w1', k, f * 128, 128, 4 * C),
                                         xh2[k][:, sl * L:(sl + 1) * L],
                                         start=(k == 0), stop=(k == 2))
                    nc.scalar.activation(fc1h[f][:, sl * L:(sl + 1) * L],
                                         psg[:, 0:L], AF.Gelu)

            t2 = [pool.tile([128, GT], BF, tag=f"t2_{f}", name=f"t2_{f}", bufs=1)
                  for f in range(3)]
            for f in range(3):
                for sl in range(G):
                    psg = ps.tile([128, 512], F32, tag="ps2", name="ps2", bufs=2)
                    for k in range(12):
                        nc.tensor.matmul(psg[:, 0:L],
                                         wslice('w2', k, f * 128, 128, C),
                                         fc1h[k][:, sl * L:(sl + 1) * L],
                                         start=(k == 0), stop=(k == 11))
                    nc.vector.tensor_tensor(t2[f][:, sl * L:(sl + 1) * L], psg[:, 0:L],
                                            t1[f][:, sl * L:(sl + 1) * L], OP.add)

            # output GEMM + relu
            t_out = [pool.tile([128, GT], BF, tag=f"to{f}", name=f"to{f}", bufs=1)
                     for f in range(2)]
            for f in range(2):
                fw = 128 if f == 0 else 64
                for sl in range(G):
                    psg = ps.tile([128, 512], F32, tag="ps2", name="ps2", bufs=2)
                    for k in range(3):
                        nc.tensor.matmul(psg[0:fw, 0:L],
                                         wslice('wo', k, f * 128, fw, OUT_DIM),
                                         t2[k][:, sl * L:(sl + 1) * L],
                                         start=(k == 0), stop=(k == 2))
                    nc.scalar.activation(t_out[f][0:fw, sl * L:(sl + 1) * L],
                                         psg[0:fw, 0:L], AF.Relu)

            # bilinear 2x upsample: premultiplies on DVE/ACT, shifted adds on
            # gpsimd tensor_tensor (its tensor_scalar ucode is pathologically
            # slow), edge copies on DVE.
            for sl in range(G):
                for f in range(2):
                    PC = 128 if f == 0 else 64
                    tin = t_out[f][0:PC, sl * L:(sl + 1) * L]
                    p25 = pool.tile([128, L], BF, tag="p25", name="p25", bufs=2)
                    p75 = pool.tile([128, L], BF, tag="p75", name="p75", bufs=2)
                    nc.scalar.activation(p25[0:PC, :], tin, AF.Copy, scale=0.25)
                    nc.vector.tensor_scalar(p75[0:PC, :], tin, 0.75, None, OP.mult)
                    XI = pool.tile([128, 2 * L], BF, tag="XI", name="XI", bufs=2)
                    vXI = XI.rearrange("p (y x t) -> p y x t", y=15, x=20)
                    v25 = p25.rearrange("p (y x o) -> p y x o", y=15, x=20)
                    v75 = p75.rearrange("p (y x o) -> p y x o", y=15, x=20)
                    vti = tin.rearrange("p (y x o) -> p y x o", y=15, x=20)
                    # even cols: out[2x] = .25 in[x-1] + .75 in[x]; edge x=0 copy
                    nc.gpsimd.tensor_tensor(vXI[0:PC, :, 1:20, 0:1], v25[0:PC, :, 0:19, :],
                                            v75[0:PC, :, 1:20, :], OP.add)
                    nc.vector.tensor_copy(vXI[0:PC, :, 0:1, 0:1], vti[0:PC, :, 0:1, :])
                    # odd cols: out[2x+1] = .75 in[x] + .25 in[x+1]; edge x=19 copy
                    nc.gpsimd.tensor_tensor(vXI[0:PC, :, 0:19, 1:2], v75[0:PC, :, 0:19, :],
                                            v25[0:PC, :, 1:20, :], OP.add)
                    nc.vector.tensor_copy(vXI[0:PC, :, 19:20, 1:2], vti[0:PC, :, 19:20, :])
                    # y-pass: EY[y] = .25 XI[y-1] + .75 XI[y]; OY[y] = .75 XI[y] + .25 XI[y+1]
                    p25y = pool.tile([128, 2 * L], BF, tag="p25y", name="p25y", bufs=2)
                    p75y = pool.tile([128, 2 * L], BF, tag="p75y", name="p75y", bufs=2)
                    nc.scalar.activation(p25y[0:PC, :], XI[0:PC, :], AF.Copy, scale=0.25)
                    nc.vector.tensor_scalar(p75y[0:PC, :], XI[0:PC, :], 0.75, None, OP.mult)
                    EY = pool.tile([128, 2 * L], BF, tag="EY", name="EY", bufs=2)
                    OY = pool.tile([128, 2 * L], BF, tag="OY", name="OY", bufs=2)
                    vEY = EY.rearrange("p (y x) -> p y x", y=15)
                    vOY = OY.rearrange("p (y x) -> p y x", y=15)
                    vq25 = p25y.rearrange("p (y x) -> p y x", y=15)
                    vq75 = p75y.rearrange("p (y x) -> p y x", y=15)
                    vXI2 = XI.rearrange("p (y x) -> p y x", y=15)
                    nc.gpsimd.tensor_tensor(vEY[0:PC, 1:15, :], vq25[0:PC, 0:14, :],
                                            vq75[0:PC, 1:15, :], OP.add)
                    nc.vector.tensor_copy(vEY[0:PC, 0:1, :], vXI2[0:PC, 0:1, :])
                    nc.gpsimd.tensor_tensor(vOY[0:PC, 0:14, :], vq75[0:PC, 0:14, :],
                                            vq25[0:PC, 1:15, :], OP.add)
                    nc.vector.tensor_copy(vOY[0:PC, 14:15, :], vXI2[0:PC, 14:15, :])
                    ov = outd[g * G + sl, f * 128:f * 128 + PC].rearrange(
                        "c (y t) x -> c y (t x)", t=2)
                    nc.sync.dma_start(ov[:, :, 0:40], vEY[0:PC, :, :])
                    nc.sync.dma_start(ov[:, :, 40:80], vOY[0:PC, :, :])

        # ================== run the 3-stage pipeline ==================
        n_groups = n_samples // G
        states = {}
        for gg in range(n_groups + 2):
            if gg < n_groups:
                states[gg] = front(gg)
            if 1 <= gg < n_groups + 1:
                mid(gg - 1, states[gg - 1])
            if gg >= 2:
                tail(gg - 2, states[gg - 2])
                del states[gg - 2]

    nc.compile()
    return nc


_PROG_CACHE = {}


def kernel(x, ln1_s, ln1_b, qkv_w, qkv_b, bias_table, proj_w, proj_b,
           ln2_s, ln2_b, fc1_w, fc1_b, fc2_w, fc2_b, out_w, out_b):
    from concourse.bass_utils import run_bass_kernel_spmd
    x = np.asarray(x, np.float32)
    consts = build_consts(ln1_s, ln1_b, qkv_w, qkv_b, bias_table, proj_w, proj_b,
                          ln2_s, ln2_b, fc1_w, fc1_b, fc2_w, fc2_b, out_w, out_b)
    if S not in _PROG_CACHE:
        _PROG_CACHE[S] = build_program(S)
    nc = _PROG_CACHE[S]
    xs = x.reshape(B, 3, 128, H * W).astype(bf16)
    in_maps = []
    for cid in range(N_CORES):
        m = {'xin': np.ascontiguousarray(xs[cid * S:(cid + 1) * S])}
        m.update(consts)
        in_maps.append(m)
    res = run_bass_kernel_spmd(nc, in_maps, core_ids=list(range(N_CORES)))
    out = np.concatenate([r['out'] for r in res.results], axis=0)
    return out.astype(np.float32)
